# revision 1
# baseline (speedup 1.0000x reference)
"""3-layer GCN + pooled MLP head on 8 Trainium2 NeuronCores.

Strategy (dst-sharded message passing):
- Relabel nodes by in-degree (desc), deal round-robin to 8 cores; each core
  owns 6250 dst nodes (padded to 6272 = 49 tiles of 128).
- Per layer: each core computes its slice of hhat = dinv * (y @ W) feature-major
  on PE, transposes to node-major, AllGathers hhat [50176, 64] to DRAM.
- Aggregation: per dst tile, dma_gather pulls 512B two-row blocks (int16 index
  = row//2) from DRAM hhat; a static 0/1 mask selects the right row parity and
  a strided DVE reduce does the per-node segment sum. Self-loop and dinv_dst
  scaling are fused in; bias+ReLU ride the PE transpose through the ACT engine.
- Head: per-core feature-major sum/max pools, tiny AllGather, replicated MLP.
"""
import os
import sys
import types

sys.path.insert(0, "/opt/trn_rl_repo")

import numpy as np

import concourse.bass as bass
import concourse.bacc as bacc
import concourse.tile as tile
import concourse.mybir as mybir
from concourse import bass_utils

N = 50000
E = 800000
D_IN = 128
H = 64
NC = 8
NPC = 6272          # padded nodes per core (49 tiles of 128)
NT = 49             # dst tiles per core
NTOT = NC * NPC     # 50176 rows in the allgathered hhat
MAXCOLS = 48        # max block-columns per dma_gather call

_EXEC_NS = [None]


def _install_trace_hook():
    try:
        from trn_agent_boot.trn_boot import _ntff_profile_via_ctypes
        hook = _ntff_profile_via_ctypes('/opt/axon/libaxon_pjrt.so')
        if hook is None:
            return False
        mod = types.ModuleType('antenv.axon_hooks')
        mod.get_axon_ntff_profile_hook = lambda: hook
        sys.modules['antenv.axon_hooks'] = mod
        return True
    except Exception:
        return False


def _preprocess(edge_index):
    """Graph partitioning: relabel, shard, tile, build gather lists + masks."""
    src = np.asarray(edge_index[0], np.int64)
    dst = np.asarray(edge_index[1], np.int64)
    deg = np.bincount(dst, minlength=N)          # in-degree (no self loop)
    dinv = (1.0 / np.sqrt(deg + 1.0)).astype(np.float32)

    order = np.argsort(-deg, kind="stable")       # relabel: rank r -> orig order[r]
    rank_of = np.empty(N, np.int64)
    rank_of[order] = np.arange(N)
    core_of = rank_of % NC
    slot_of = rank_of // NC                       # 0..6249, degree-desc within core
    row_of = core_of * NPC + slot_of              # DRAM row in hhat_full

    per_core = []
    src_row = row_of[src]
    dst_core = core_of[dst]
    dst_slot = slot_of[dst]
    for c in range(NC):
        em = dst_core == c
        e_slot = dst_slot[em]
        e_srow = src_row[em]
        # neighbor lists per local slot
        o = np.argsort(e_slot, kind="stable")
        e_slot = e_slot[o]
        e_srow = e_srow[o]
        counts = np.bincount(e_slot, minlength=NPC)
        starts = np.concatenate([[0], np.cumsum(counts)])
        tiles = []
        for t in range(NT):
            sl0 = t * 128
            nt = int(counts[sl0:sl0 + 128].max()) if counts[sl0:sl0 + 128].size else 0
            if nt == 0:
                tiles.append((0, None, None))
                continue
            idx = np.zeros((nt, 128), np.int16)       # [col, part] block ids
            mask = np.zeros((128, nt, 2), np.float32)  # [part, col, parity]
            for p in range(128):
                s = sl0 + p
                if s >= NPC:
                    continue
                rows = e_srow[starts[s]:starts[s + 1]]
                k = len(rows)
                if k:
                    idx[:k, p] = (rows // 2).astype(np.int16)
                    mask[p, np.arange(k), (rows % 2)] = 1.0
            tiles.append((nt, idx, mask))
        per_core.append(tiles)
    return dinv, order, core_of, slot_of, per_core


def _build_core_inputs(x, dinv, order, per_core_tiles, c):
    """Per-core numpy inputs."""
    ranks = np.arange(c, N, NC)                  # global ranks owned by core c
    orig = order[ranks]                          # original node ids, slot order
    xT = np.zeros((D_IN, NPC), np.float32)
    xT[:, :len(orig)] = np.asarray(x, np.float32)[orig].T
    dv = np.zeros(NPC, np.float32)
    dv[:len(orig)] = dinv[orig]
    dinv_col = dv.reshape(NT, 128).T.copy()      # [128, 49]

    tiles = per_core_tiles[c]
    idx_parts, mask_parts, groups = [], [], []
    g_cols, g_tiles = 0, []
    for t in range(NT):
        nt = tiles[t][0]
        if g_tiles and g_cols + nt > MAXCOLS:
            groups.append(g_tiles)
            g_tiles, g_cols = [], 0
        g_tiles.append(t)
        g_cols += nt
    if g_tiles:
        groups.append(g_tiles)

    for g_tiles in groups:
        lst = []
        for t in g_tiles:
            nt, idx, mask = tiles[t]
            if nt:
                lst.append(idx.reshape(-1))       # [col, part] -> flat c*128+p
                mask_parts.append(mask.reshape(128, nt * 2))
        if lst:
            idx_parts.append(np.concatenate(lst))
    flat = np.concatenate(idx_parts) if idx_parts else np.zeros(128, np.int16)
    ncols_tot = len(flat) // 128
    wrap = np.zeros((128, len(flat) // 16), np.int16)
    a = flat.reshape(-1, 16)
    for gg in range(8):
        wrap[gg * 16:(gg + 1) * 16, :] = a.T
    maskcat = (np.concatenate(mask_parts, axis=1) if mask_parts
               else np.zeros((128, 2), np.float32))
    return xT, dinv_col, wrap, maskcat, groups, ncols_tot


def _build_program(tiles_meta, groups_meta, idx_cols16, mask_cols):
    """Build the bass program (same for all cores; per-core data via inputs)."""
    nc = bacc.Bacc("TRN2", target_bir_lowering=False, debug=False, num_devices=NC)
    f32 = mybir.dt.float32
    xT_d = nc.dram_tensor("xT", [D_IN, NPC], f32, kind="ExternalInput")
    W1_d = nc.dram_tensor("W1", [D_IN, H], f32, kind="ExternalInput")
    W2_d = nc.dram_tensor("W2", [H, H], f32, kind="ExternalInput")
    W3_d = nc.dram_tensor("W3", [H, H], f32, kind="ExternalInput")
    bcol_d = nc.dram_tensor("bcol", [H, 3], f32, kind="ExternalInput")
    dinvc_d = nc.dram_tensor("dinvc", [128, NT], f32, kind="ExternalInput")
    idx_d = nc.dram_tensor("idx16", [128, idx_cols16], mybir.dt.int16, kind="ExternalInput")
    mask_d = nc.dram_tensor("maskc", [128, mask_cols], f32, kind="ExternalInput")
    ident_d = nc.dram_tensor("ident", [128, 128], f32, kind="ExternalInput")
    fw1_d = nc.dram_tensor("fw1", [2 * H, H], f32, kind="ExternalInput")
    fb1_d = nc.dram_tensor("fb1", [H, 1], f32, kind="ExternalInput")
    fw2_d = nc.dram_tensor("fw2", [H, 1], f32, kind="ExternalInput")
    fb2_d = nc.dram_tensor("fb2", [1, 1], f32, kind="ExternalInput")
    out_d = nc.dram_tensor("out", [1, 1], f32, kind="ExternalOutput")

    Ws = [W1_d, W2_d, W3_d]
    NCHUNK = NPC // 512  # 12.25 -> handle per 512 with last 128-only tail via tiles

    with tile.TileContext(nc) as tc:
        with (
            tc.tile_pool(name="const", bufs=1) as cst,
            tc.tile_pool(name="hhat", bufs=1) as hhp,
            tc.tile_pool(name="yt", bufs=1) as ytp,
            tc.tile_pool(name="gb", bufs=2) as gbp,
            tc.tile_pool(name="acc", bufs=3) as accp,
            tc.tile_pool(name="ps", bufs=2, space="PSUM") as psp,
            tc.tile_pool(name="hdps", bufs=1, space="PSUM") as hdp,
            tc.tile_pool(name="zps", bufs=2, space="PSUM") as zpsp,
            tc.tile_pool(name="zsb", bufs=2) as zsbp,
            tc.tile_pool(name="dram", bufs=1, space="DRAM") as dram,
        ):
            # constants
            xT = cst.tile([D_IN, NPC], f32)
            nc.sync.dma_start(out=xT[:], in_=xT_d[:])
            W1 = cst.tile([D_IN, H], f32)
            nc.sync.dma_start(out=W1[:], in_=W1_d[:])
            W2 = cst.tile([H, H], f32)
            nc.sync.dma_start(out=W2[:], in_=W2_d[:])
            W3 = cst.tile([H, H], f32)
            nc.sync.dma_start(out=W3[:], in_=W3_d[:])
            Wt = [W1, W2, W3]
            bcol = cst.tile([H, 3], f32)
            nc.sync.dma_start(out=bcol[:], in_=bcol_d[:])
            dinvc = cst.tile([128, NT], f32)
            nc.sync.dma_start(out=dinvc[:], in_=dinvc_d[:])
            idx16 = cst.tile([128, idx_cols16], mybir.dt.int16)
            nc.sync.dma_start(out=idx16[:], in_=idx_d[:])
            maskc = cst.tile([128, mask_cols], f32)
            nc.sync.dma_start(out=maskc[:], in_=mask_d[:])
            ident = cst.tile([128, 128], f32)
            nc.sync.dma_start(out=ident[:], in_=ident_d[:])
            fw1 = cst.tile([2 * H, H], f32)
            nc.sync.dma_start(out=fw1[:], in_=fw1_d[:])
            fb1 = cst.tile([H, 1], f32)
            nc.sync.dma_start(out=fb1[:], in_=fb1_d[:])
            fw2 = cst.tile([H, 1], f32)
            nc.sync.dma_start(out=fw2[:], in_=fw2_d[:])
            fb2 = cst.tile([1, 1], f32)
            nc.sync.dma_start(out=fb2[:], in_=fb2_d[:])

            hhat = hhp.tile([128, NT * H], f32)       # node-major own hhat
            yT = ytp.tile([H, NPC], f32)              # feature-major relu output

            ag_in = [dram.tile([NPC, H], f32, name=f"agin{l}") for l in range(3)]
            ag_out = [dram.tile([NTOT, H], f32, addr_space="Shared", name=f"agout{l}")
                      for l in range(3)]

            def mm_rows(l, rhs_sb, rhs_cols):
                """z_T = W_l^T @ rhs over column chunks; transpose+scale to hhat;
                write hhat -> ag_in[l]; launch AllGather."""
                for ch0 in range(0, rhs_cols, 512):
                    cw = min(512, rhs_cols - ch0)
                    zps = zpsp.tile([H, 512], f32, tag="zps")
                    nc.tensor.matmul(out=zps[:, :cw], lhsT=Wt[l][:],
                                     rhs=rhs_sb[:, ch0:ch0 + cw],
                                     start=True, stop=True)
                    zsb = zsbp.tile([H, 512], f32, tag="zsb")
                    nc.vector.tensor_copy(out=zsb[:, :cw], in_=zps[:, :cw])
                    for q in range(0, cw, 128):
                        t = (ch0 + q) // 128
                        tp = psp.tile([128, H], f32, tag="tp")
                        nc.tensor.transpose(out=tp[:], in_=zsb[:, q:q + 128],
                                            identity=ident[:H, :H])
                        nc.vector.tensor_scalar(
                            out=hhat[:, t * H:(t + 1) * H], in0=tp[:],
                            scalar1=dinvc[:, t:t + 1], scalar2=None,
                            op0=mybir.AluOpType.mult)
                nc.sync.dma_start(
                    out=ag_in[l][:].rearrange("(t p) d -> p t d", p=128),
                    in_=hhat[:].rearrange("p (t d) -> p t d", d=H))
                nc.gpsimd.collective_compute(
                    "AllGather", mybir.AluOpType.bypass,
                    replica_groups=[list(range(NC))],
                    ins=[ag_in[l].opt()], outs=[ag_out[l].opt()])

            def aggregate(l):
                """dst-tile aggregation from ag_out[l] into yT (or return y3)."""
                src_view = ag_out[l][:].rearrange("(a b) d -> a (b d)", b=2)
                cs_off = 0
                col_off = 0
                for g_tiles in groups_meta:
                    cols = sum(tiles_meta[t][0] for t in g_tiles)
                    if cols == 0:
                        continue
                    nidx = cols * 128
                    gb = gbp.tile([128, cols * 128], f32, tag="gb",
                                  name=f"gb{l}_{g_tiles[0]}")
                    nc.gpsimd.dma_gather(
                        out_ap=gb[:].rearrange("p (n d) -> p n d", d=128),
                        in_ap=src_view,
                        idxs_ap=idx16[:, col_off * 8:(col_off + cols) * 8],
                        num_idxs=nidx, num_idxs_reg=nidx,
                        elem_size=128, single_packet=False)
                    seg = 0
                    for t in g_tiles:
                        nt = tiles_meta[t][0]
                        if nt == 0:
                            continue
                        gseg = gb[:, seg * 128:(seg + nt) * 128]
                        mseg = maskc[:, cs_off:cs_off + nt * 2]
                        nc.vector.tensor_tensor(
                            out=gseg.rearrange("p (cs d) -> p cs d", d=H),
                            in0=gseg.rearrange("p (cs d) -> p cs d", d=H),
                            in1=mseg.rearrange("p (cs u) -> p cs u", u=1)
                                .to_broadcast([128, nt * 2, H]),
                            op=mybir.AluOpType.mult)
                        acc = accp.tile([128, H], f32, tag="acc", name=f"acc{l}_{t}")
                        nc.vector.tensor_reduce(
                            out=acc[:],
                            in_=gseg.rearrange("p (cs d) -> p d cs", d=H),
                            axis=mybir.AxisListType.X, op=mybir.AluOpType.add)
                        nc.vector.tensor_add(out=acc[:], in0=acc[:],
                                             in1=hhat[:, t * H:(t + 1) * H])
                        nc.vector.tensor_scalar(
                            out=acc[:], in0=acc[:], scalar1=dinvc[:, t:t + 1],
                            scalar2=None, op0=mybir.AluOpType.mult)
                        yps = psp.tile([H, 128], f32, tag="yps", name=f"yps{l}_{t}")
                        nc.tensor.transpose(out=yps[:], in_=acc[:], identity=ident[:])
                        nc.scalar.activation(
                            out=yT[:, t * 128:(t + 1) * 128], in_=yps[:],
                            func=mybir.ActivationFunctionType.Relu,
                            bias=bcol[:, l:l + 1])
                        cs_off += nt * 2
                        seg += nt
                    col_off += cols

            # ---- layer 1 ----
            mm_rows(0, xT, NPC)
            aggregate(0)
            # ---- layer 2 ----
            mm_rows(1, yT, NPC)
            aggregate(1)
            # ---- layer 3 ----
            mm_rows(2, yT, NPC)
            aggregate(2)

            # zero pad columns of y3 (slots 6250..6271) before pooling
            nc.vector.memset(yT[:, NPC - 22:], 0.0)

            # pooling: sum and max over own nodes, feature-major
            sum_acc = accp.tile([H, 1], f32, tag="pool", name="sum_acc")
            max_acc = accp.tile([H, 1], f32, tag="pool", name="max_acc")
            first = True
            for ch0 in range(0, NPC, 512):
                cw = min(512, NPC - ch0)
                rs = accp.tile([H, 1], f32, tag="pool", name=f"rs{ch0}")
                rm = accp.tile([H, 1], f32, tag="pool", name=f"rm{ch0}")
                nc.vector.reduce_sum(out=rs[:], in_=yT[:, ch0:ch0 + cw],
                                     axis=mybir.AxisListType.X)
                nc.vector.reduce_max(out=rm[:], in_=yT[:, ch0:ch0 + cw],
                                     axis=mybir.AxisListType.X)
                if first:
                    nc.vector.tensor_copy(out=sum_acc[:], in_=rs[:])
                    nc.vector.tensor_copy(out=max_acc[:], in_=rm[:])
                    first = False
                else:
                    nc.vector.tensor_add(out=sum_acc[:], in0=sum_acc[:], in1=rs[:])
                    nc.vector.tensor_max(out=max_acc[:], in0=max_acc[:], in1=rm[:])

            pool2 = accp.tile([H, 2], f32, tag="pool", name="pool2")
            nc.vector.tensor_copy(out=pool2[:, 0:1], in_=sum_acc[:])
            nc.vector.tensor_copy(out=pool2[:, 1:2], in_=max_acc[:])
            agp_in = dram.tile([H, 2], f32, name="agpin")
            agp_out = dram.tile([NC * H, 2], f32, addr_space="Shared", name="agpout")
            nc.sync.dma_start(out=agp_in[:], in_=pool2[:])
            nc.gpsimd.collective_compute(
                "AllGather", mybir.AluOpType.bypass,
                replica_groups=[list(range(NC))],
                ins=[agp_in.opt()], outs=[agp_out.opt()])
            allp = accp.tile([H, 2 * NC], f32, tag="allp", name="allp")
            # rank r lands at cols 2r:2r+2 -> [64, 16]
            nc.sync.dma_start(
                out=allp[:].rearrange("p (r d) -> p r d", d=2),
                in_=agp_out[:].rearrange("(r p) d -> p r d", p=H))
            gsum = accp.tile([H, 1], f32, tag="pool", name="gsum")
            gmax = accp.tile([H, 1], f32, tag="pool", name="gmax")
            nc.vector.reduce_sum(
                out=gsum[:], in_=allp[:].rearrange("p (r d) -> p d r", d=2)[:, 0:1, :],
                axis=mybir.AxisListType.X)
            nc.vector.reduce_max(
                out=gmax[:], in_=allp[:].rearrange("p (r d) -> p d r", d=2)[:, 1:2, :],
                axis=mybir.AxisListType.X)
            nc.vector.tensor_scalar(out=gsum[:], in0=gsum[:], scalar1=1.0 / N,
                                    scalar2=None, op0=mybir.AluOpType.mult)
            pooled = accp.tile([2 * H, 1], f32, tag="pooled", name="pooled")
            nc.sync.dma_start(out=pooled[:H, :], in_=gsum[:])
            nc.sync.dma_start(out=pooled[H:, :], in_=gmax[:])
            h1ps = hdp.tile([H, 1], f32, tag="hd", name="h1ps")
            nc.tensor.matmul(out=h1ps[:], lhsT=fw1[:], rhs=pooled[:],
                             start=True, stop=True)
            r1 = accp.tile([H, 1], f32, tag="pool", name="r1")
            nc.scalar.activation(out=r1[:], in_=h1ps[:],
                                 func=mybir.ActivationFunctionType.Relu,
                                 bias=fb1[:, 0:1])
            h2ps = hdp.tile([1, 1], f32, tag="hd2", name="h2ps")
            nc.tensor.matmul(out=h2ps[:], lhsT=fw2[:], rhs=r1[:],
                             start=True, stop=True)
            ores = accp.tile([1, 1], f32, tag="ores", name="ores")
            nc.vector.tensor_add(out=ores[:], in0=h2ps[:],
                                 in1=fb2[:, 0:1])
            nc.sync.dma_start(out=out_d[:], in_=ores[:])

    nc.compile()
    return nc


def kernel(x, edge_index, W1, b1, W2, b2, W3, b3, fw1, fb1, fw2, fb2):
    dinv, order, core_of, slot_of, per_core = _preprocess(edge_index)

    core_inputs = []
    tiles_meta = None
    groups_meta = None
    idx_cols16 = mask_cols = None
    per_core_np = []
    for c in range(NC):
        xT, dinv_col, wrap, maskcat, groups, ncols = _build_core_inputs(
            x, dinv, order, per_core, c)
        per_core_np.append((xT, dinv_col, wrap, maskcat, groups, ncols))

    # All cores must share ONE program => unify tile/group metadata by padding
    # per-tile column counts to the max across cores.
    nts = np.zeros((NC, NT), np.int64)
    for c in range(NC):
        for t in range(NT):
            nts[c, t] = per_core[c][t][0]
    nt_max = nts.max(axis=0)

    # rebuild per-core inputs with unified nt per tile
    tiles_meta = [(int(nt_max[t]), None, None) for t in range(NT)]
    groups_meta = []
    g_tiles, g_cols = [], 0
    for t in range(NT):
        nt = int(nt_max[t])
        if g_tiles and g_cols + nt > MAXCOLS:
            groups_meta.append(g_tiles)
            g_tiles, g_cols = [], 0
        g_tiles.append(t)
        g_cols += nt
    if g_tiles:
        groups_meta.append(g_tiles)

    total_cols = int(nt_max.sum())
    idx_cols16 = total_cols * 8
    mask_cols = total_cols * 2

    in_maps = []
    b3col = np.stack([np.asarray(b1, np.float32), np.asarray(b2, np.float32),
                      np.asarray(b3, np.float32)], axis=1)  # [64, 3]
    ident = np.eye(128, dtype=np.float32)
    for c in range(NC):
        tiles = per_core[c]
        idx_flat = []
        mask_parts = []
        for g in groups_meta:
            for t in g:
                ntu = int(nt_max[t])
                if ntu == 0:
                    continue
                nt, idx, mask = tiles[t]
                idxu = np.zeros((ntu, 128), np.int16)
                masku = np.zeros((128, ntu, 2), np.float32)
                if nt:
                    idxu[:nt] = idx
                    masku[:, :nt, :] = mask
                idx_flat.append(idxu.reshape(-1))
                mask_parts.append(masku.reshape(128, ntu * 2))
        flat = np.concatenate(idx_flat)
        wrap = np.zeros((128, len(flat) // 16), np.int16)
        a = flat.reshape(-1, 16)
        for gg in range(8):
            wrap[gg * 16:(gg + 1) * 16, :] = a.T
        maskcat = np.concatenate(mask_parts, axis=1)
        xT, dinv_col = per_core_np[c][0], per_core_np[c][1]
        in_maps.append({
            "xT": xT, "W1": np.asarray(W1, np.float32),
            "W2": np.asarray(W2, np.float32), "W3": np.asarray(W3, np.float32),
            "bcol": b3col, "dinvc": dinv_col, "idx16": wrap, "maskc": maskcat,
            "ident": ident, "fw1": np.asarray(fw1, np.float32),
            "fb1": np.asarray(fb1, np.float32).reshape(H, 1),
            "fw2": np.asarray(fw2, np.float32).reshape(H, 1),
            "fb2": np.asarray(fb2, np.float32).reshape(1, 1),
        })

    nc = _build_program(tiles_meta, groups_meta, idx_cols16, mask_cols)

    trace = os.environ.get("BASS_GCN_TRACE", "0") == "1"
    if trace:
        trace = _install_trace_hook()
    res = bass_utils.run_bass_kernel_spmd(
        nc, in_maps, core_ids=list(range(NC)), trace=trace)
    _EXEC_NS[0] = res.exec_time_ns
    out = res.results[0]["out"]
    return np.asarray(out, np.float32).reshape(1, 1)



# revision 10
# speedup vs baseline: 1.0178x; 1.0178x over previous
"""3-layer GCN + pooled MLP head on 8 Trainium2 NeuronCores.

Strategy (dst-sharded message passing):
- Relabel nodes by in-degree (desc), deal round-robin to 8 cores; each core
  owns 6250 dst nodes (padded to 6272 = 49 tiles of 128).
- Per layer: each core computes its slice of hhat = dinv * (y @ W) feature-major
  on PE, transposes to node-major, AllGathers hhat [50176, 64] to DRAM.
- Aggregation: per dst tile, dma_gather pulls 512B two-row blocks (int16 index
  = row//2) from DRAM hhat; a static 0/1 mask selects the right row parity and
  a strided DVE reduce does the per-node segment sum. Self-loop and dinv_dst
  scaling are fused in; bias+ReLU ride the PE transpose through the ACT engine.
- Head: per-core feature-major sum/max pools, tiny AllGather, replicated MLP.
"""
import os
import sys
import types

sys.path.insert(0, "/opt/trn_rl_repo")

import ml_dtypes
import numpy as np

import concourse.bass as bass
import concourse.bacc as bacc
import concourse.tile as tile
import concourse.mybir as mybir
from concourse import bass_utils

N = 50000
E = 800000
D_IN = 128
H = 64
NC = 8
NPC = 6272          # padded nodes per core (49 tiles of 128)
NT = 49             # dst tiles per core
NTOT = NC * NPC     # 50176 rows in the allgathered hhat
MAXCOLS = 48        # max block-columns per dma_gather call

_EXEC_NS = [None]


def _install_trace_hook():
    try:
        from trn_agent_boot.trn_boot import _ntff_profile_via_ctypes
        hook = _ntff_profile_via_ctypes('/opt/axon/libaxon_pjrt.so')
        if hook is None:
            return False
        mod = types.ModuleType('antenv.axon_hooks')
        mod.get_axon_ntff_profile_hook = lambda: hook
        sys.modules['antenv.axon_hooks'] = mod
        return True
    except Exception:
        return False


def _preprocess(edge_index):
    """Graph partitioning: relabel, shard, tile, build gather lists + masks."""
    src = np.asarray(edge_index[0], np.int64)
    dst = np.asarray(edge_index[1], np.int64)
    deg = np.bincount(dst, minlength=N)          # in-degree (no self loop)
    dinv = (1.0 / np.sqrt(deg + 1.0)).astype(np.float32)

    order = np.argsort(-deg, kind="stable")       # relabel: rank r -> orig order[r]
    rank_of = np.empty(N, np.int64)
    rank_of[order] = np.arange(N)
    core_of = rank_of % NC
    slot_of = rank_of // NC                       # 0..6249, degree-desc within core
    row_of = core_of * NPC + slot_of              # DRAM row in hhat_full

    per_core = []
    src_row = row_of[src]
    dst_core = core_of[dst]
    dst_slot = slot_of[dst]
    for c in range(NC):
        em = dst_core == c
        e_slot = dst_slot[em]
        e_srow = src_row[em]
        # neighbor lists per local slot
        o = np.argsort(e_slot, kind="stable")
        e_slot = e_slot[o]
        e_srow = e_srow[o]
        counts = np.bincount(e_slot, minlength=NPC)
        starts = np.concatenate([[0], np.cumsum(counts)])
        tiles = []
        for t in range(NT):
            sl0 = t * 128
            nt = int(counts[sl0:sl0 + 128].max()) if counts[sl0:sl0 + 128].size else 0
            if nt == 0:
                tiles.append((0, None, None))
                continue
            idx = np.zeros((nt, 128), np.int16)       # [col, part] block ids
            mask = np.zeros((128, nt, 2), np.float32)  # [part, col, parity]
            for p in range(128):
                s = sl0 + p
                if s >= NPC:
                    continue
                rows = e_srow[starts[s]:starts[s + 1]]
                k = len(rows)
                if k:
                    idx[:k, p] = (rows // 2).astype(np.int16)
                    mask[p, np.arange(k), (rows % 2)] = 1.0
            tiles.append((nt, idx, mask))
        per_core.append(tiles)
    return dinv, order, core_of, slot_of, per_core


def _build_core_inputs(x, dinv, order, per_core_tiles, c):
    """Per-core numpy inputs."""
    ranks = np.arange(c, N, NC)                  # global ranks owned by core c
    orig = order[ranks]                          # original node ids, slot order
    xT = np.zeros((D_IN, NPC), np.float32)
    xT[:, :len(orig)] = np.asarray(x, np.float32)[orig].T
    dv = np.zeros(NPC, np.float32)
    dv[:len(orig)] = dinv[orig]
    dinv_col = dv.reshape(NT, 128).T.copy()      # [128, 49]

    tiles = per_core_tiles[c]
    idx_parts, mask_parts, groups = [], [], []
    g_cols, g_tiles = 0, []
    for t in range(NT):
        nt = tiles[t][0]
        if g_tiles and g_cols + nt > MAXCOLS:
            groups.append(g_tiles)
            g_tiles, g_cols = [], 0
        g_tiles.append(t)
        g_cols += nt
    if g_tiles:
        groups.append(g_tiles)

    for g_tiles in groups:
        lst = []
        for t in g_tiles:
            nt, idx, mask = tiles[t]
            if nt:
                lst.append(idx.reshape(-1))       # [col, part] -> flat c*128+p
                mask_parts.append(mask.reshape(128, nt * 2))
        if lst:
            idx_parts.append(np.concatenate(lst))
    flat = np.concatenate(idx_parts) if idx_parts else np.zeros(128, np.int16)
    ncols_tot = len(flat) // 128
    wrap = np.zeros((128, len(flat) // 16), np.int16)
    a = flat.reshape(-1, 16)
    for gg in range(8):
        wrap[gg * 16:(gg + 1) * 16, :] = a.T
    maskcat = (np.concatenate(mask_parts, axis=1) if mask_parts
               else np.zeros((128, 2), np.float32))
    return xT, dinv_col, wrap, maskcat, groups, ncols_tot


def _build_program(tiles_meta, groups_meta, idx_cols16, mask_cols):
    """Build the bass program (same for all cores; per-core data via inputs)."""
    nc = bacc.Bacc("TRN2", target_bir_lowering=False, debug=False, num_devices=NC)
    f32 = mybir.dt.float32
    bf16 = mybir.dt.bfloat16
    xT_d = nc.dram_tensor("xT", [D_IN, NPC], f32, kind="ExternalInput")
    W1_d = nc.dram_tensor("W1", [D_IN, H], f32, kind="ExternalInput")
    W2_d = nc.dram_tensor("W2", [H, H], f32, kind="ExternalInput")
    W3_d = nc.dram_tensor("W3", [H, H], f32, kind="ExternalInput")
    bcol_d = nc.dram_tensor("bcol", [H, 3], f32, kind="ExternalInput")
    dinvc_d = nc.dram_tensor("dinvc", [128, NT], f32, kind="ExternalInput")
    idx_d = nc.dram_tensor("idx16", [128, idx_cols16], mybir.dt.int16, kind="ExternalInput")
    mask_d = nc.dram_tensor("maskc", [128, mask_cols], mybir.dt.bfloat16,
                            kind="ExternalInput")
    ident_d = nc.dram_tensor("ident", [128, 128], f32, kind="ExternalInput")
    fw1_d = nc.dram_tensor("fw1", [2 * H, H], f32, kind="ExternalInput")
    fb1_d = nc.dram_tensor("fb1", [H, 1], f32, kind="ExternalInput")
    fw2_d = nc.dram_tensor("fw2", [H, 1], f32, kind="ExternalInput")
    fb2_d = nc.dram_tensor("fb2", [1, 1], f32, kind="ExternalInput")
    out_d = nc.dram_tensor("out", [1, 1], f32, kind="ExternalOutput")

    Ws = [W1_d, W2_d, W3_d]
    NCHUNK = NPC // 512  # 12.25 -> handle per 512 with last 128-only tail via tiles

    with tile.TileContext(nc) as tc:
        with (
            tc.tile_pool(name="const", bufs=1) as cst,
            tc.tile_pool(name="hhat", bufs=1) as hhp,
            tc.tile_pool(name="yt", bufs=1) as ytp,
            tc.tile_pool(name="gb", bufs=2) as gbp,
            tc.tile_pool(name="acc", bufs=3) as accp,
            tc.tile_pool(name="ps", bufs=2, space="PSUM") as psp,
            tc.tile_pool(name="hdps", bufs=1, space="PSUM") as hdp,
            tc.tile_pool(name="zps", bufs=2, space="PSUM") as zpsp,
            tc.tile_pool(name="zsb", bufs=2) as zsbp,
            tc.tile_pool(name="dram", bufs=1, space="DRAM") as dram,
        ):
            # constants
            xT = cst.tile([D_IN, NPC], f32)
            nc.sync.dma_start(out=xT[:], in_=xT_d[:])
            W1 = cst.tile([D_IN, H], f32)
            nc.sync.dma_start(out=W1[:], in_=W1_d[:])
            W2 = cst.tile([H, H], f32)
            nc.sync.dma_start(out=W2[:], in_=W2_d[:])
            W3 = cst.tile([H, H], f32)
            nc.sync.dma_start(out=W3[:], in_=W3_d[:])
            Wt = [W1, W2, W3]
            bcol = cst.tile([H, 3], f32)
            nc.sync.dma_start(out=bcol[:], in_=bcol_d[:])
            dinvc = cst.tile([128, NT], f32)
            nc.sync.dma_start(out=dinvc[:], in_=dinvc_d[:])
            idx16 = cst.tile([128, idx_cols16], mybir.dt.int16)
            nc.sync.dma_start(out=idx16[:], in_=idx_d[:])
            maskc = cst.tile([128, mask_cols], bf16)
            nc.sync.dma_start(out=maskc[:], in_=mask_d[:])
            ident = cst.tile([128, 128], f32)
            nc.sync.dma_start(out=ident[:], in_=ident_d[:])
            fw1 = cst.tile([2 * H, H], f32)
            nc.sync.dma_start(out=fw1[:], in_=fw1_d[:])
            fb1 = cst.tile([H, 1], f32)
            nc.sync.dma_start(out=fb1[:], in_=fb1_d[:])
            fw2 = cst.tile([H, 1], f32)
            nc.sync.dma_start(out=fw2[:], in_=fw2_d[:])
            fb2 = cst.tile([1, 1], f32)
            nc.sync.dma_start(out=fb2[:], in_=fb2_d[:])

            hhat = hhp.tile([128, NT * H], f32)       # node-major own hhat
            hhat_bf = hhp.tile([128, NT * H], bf16)   # bf16 copy for allgather
            yT = ytp.tile([H, NPC], f32)              # feature-major relu output

            ag_in = [dram.tile([NPC, H], bf16, name=f"agin{l}") for l in range(3)]
            ag_out = [dram.tile([NTOT, H], bf16, addr_space="Shared", name=f"agout{l}")
                      for l in range(3)]

            def mm_rows(l, rhs_sb, rhs_cols):
                """z_T = W_l^T @ rhs over column chunks; transpose+scale to hhat;
                write hhat -> ag_in[l]; launch AllGather."""
                for ch0 in range(0, rhs_cols, 512):
                    cw = min(512, rhs_cols - ch0)
                    zps = zpsp.tile([H, 512], f32, tag="zps")
                    nc.tensor.matmul(out=zps[:, :cw], lhsT=Wt[l][:],
                                     rhs=rhs_sb[:, ch0:ch0 + cw],
                                     start=True, stop=True)
                    zsb = zsbp.tile([H, 512], f32, tag="zsb")
                    nc.vector.tensor_copy(out=zsb[:, :cw], in_=zps[:, :cw])
                    for q in range(0, cw, 128):
                        t = (ch0 + q) // 128
                        tp = psp.tile([128, H], f32, tag="tp")
                        nc.tensor.transpose(out=tp[:], in_=zsb[:, q:q + 128],
                                            identity=ident[:H, :H])
                        nc.vector.tensor_scalar(
                            out=hhat[:, t * H:(t + 1) * H], in0=tp[:],
                            scalar1=dinvc[:, t:t + 1], scalar2=None,
                            op0=mybir.AluOpType.mult)
                        nc.vector.tensor_copy(
                            out=hhat_bf[:, t * H:(t + 1) * H],
                            in_=hhat[:, t * H:(t + 1) * H])
                nc.sync.dma_start(
                    out=ag_in[l][:].rearrange("(t p) d -> p t d", p=128),
                    in_=hhat_bf[:].rearrange("p (t d) -> p t d", d=H))
                nc.gpsimd.collective_compute(
                    "AllGather", mybir.AluOpType.bypass,
                    replica_groups=[list(range(NC))],
                    ins=[ag_in[l].opt()], outs=[ag_out[l].opt()])

            def aggregate(l):
                """dst-tile aggregation from ag_out[l] into yT (or return y3)."""
                src_view = ag_out[l][:].rearrange("(a b) d -> a (b d)", b=2)
                cs_off = 0
                col_off = 0
                for g_tiles in groups_meta:
                    cols = sum(tiles_meta[t][0] for t in g_tiles)
                    if cols == 0:
                        continue
                    nidx = cols * 128
                    gb = gbp.tile([128, cols * 128], bf16, tag="gb",
                                  name=f"gb{l}_{g_tiles[0]}")
                    nc.gpsimd.dma_gather(
                        out_ap=gb[:].rearrange("p (n d) -> p n d", d=128),
                        in_ap=src_view,
                        idxs_ap=idx16[:, col_off * 8:(col_off + cols) * 8],
                        num_idxs=nidx, num_idxs_reg=nidx,
                        elem_size=128, single_packet=False)
                    seg = 0
                    for t in g_tiles:
                        nt = tiles_meta[t][0]
                        if nt == 0:
                            continue
                        gseg = gb[:, seg * 128:(seg + nt) * 128]
                        mseg = maskc[:, cs_off:cs_off + nt * 2]
                        nc.vector.tensor_tensor(
                            out=gseg.rearrange("p (cs d) -> p cs d", d=H),
                            in0=gseg.rearrange("p (cs d) -> p cs d", d=H),
                            in1=mseg.rearrange("p (cs u) -> p cs u", u=1)
                                .to_broadcast([128, nt * 2, H]),
                            op=mybir.AluOpType.mult)
                        acc = accp.tile([128, H], f32, tag="acc", name=f"acc{l}_{t}")
                        nc.vector.tensor_reduce(
                            out=acc[:],
                            in_=gseg.rearrange("p (cs d) -> p d cs", d=H),
                            axis=mybir.AxisListType.X, op=mybir.AluOpType.add)
                        nc.vector.tensor_add(out=acc[:], in0=acc[:],
                                             in1=hhat[:, t * H:(t + 1) * H])
                        nc.vector.tensor_scalar(
                            out=acc[:], in0=acc[:], scalar1=dinvc[:, t:t + 1],
                            scalar2=None, op0=mybir.AluOpType.mult)
                        yps = psp.tile([H, 128], f32, tag="yps", name=f"yps{l}_{t}")
                        nc.tensor.transpose(out=yps[:], in_=acc[:], identity=ident[:])
                        nc.scalar.activation(
                            out=yT[:, t * 128:(t + 1) * 128], in_=yps[:],
                            func=mybir.ActivationFunctionType.Relu,
                            bias=bcol[:, l:l + 1])
                        cs_off += nt * 2
                        seg += nt
                    col_off += cols

            # ---- layer 1 ----
            mm_rows(0, xT, NPC)
            aggregate(0)
            # ---- layer 2 ----
            mm_rows(1, yT, NPC)
            aggregate(1)
            # ---- layer 3 ----
            mm_rows(2, yT, NPC)
            aggregate(2)

            # zero pad columns of y3 (slots 6250..6271) before pooling
            nc.vector.memset(yT[:, NPC - 22:], 0.0)

            # pooling: sum and max over own nodes, feature-major
            sum_acc = accp.tile([H, 1], f32, tag="pool", name="sum_acc")
            max_acc = accp.tile([H, 1], f32, tag="pool", name="max_acc")
            first = True
            for ch0 in range(0, NPC, 512):
                cw = min(512, NPC - ch0)
                rs = accp.tile([H, 1], f32, tag="pool", name=f"rs{ch0}")
                rm = accp.tile([H, 1], f32, tag="pool", name=f"rm{ch0}")
                nc.vector.reduce_sum(out=rs[:], in_=yT[:, ch0:ch0 + cw],
                                     axis=mybir.AxisListType.X)
                nc.vector.reduce_max(out=rm[:], in_=yT[:, ch0:ch0 + cw],
                                     axis=mybir.AxisListType.X)
                if first:
                    nc.vector.tensor_copy(out=sum_acc[:], in_=rs[:])
                    nc.vector.tensor_copy(out=max_acc[:], in_=rm[:])
                    first = False
                else:
                    nc.vector.tensor_add(out=sum_acc[:], in0=sum_acc[:], in1=rs[:])
                    nc.vector.tensor_max(out=max_acc[:], in0=max_acc[:], in1=rm[:])

            pool2 = accp.tile([H, 2], f32, tag="pool", name="pool2")
            nc.vector.tensor_copy(out=pool2[:, 0:1], in_=sum_acc[:])
            nc.vector.tensor_copy(out=pool2[:, 1:2], in_=max_acc[:])
            agp_in = dram.tile([H, 2], f32, name="agpin")
            agp_out = dram.tile([NC * H, 2], f32, addr_space="Shared", name="agpout")
            nc.sync.dma_start(out=agp_in[:], in_=pool2[:])
            nc.gpsimd.collective_compute(
                "AllGather", mybir.AluOpType.bypass,
                replica_groups=[list(range(NC))],
                ins=[agp_in.opt()], outs=[agp_out.opt()])
            allp = accp.tile([H, 2 * NC], f32, tag="allp", name="allp")
            # rank r lands at cols 2r:2r+2 -> [64, 16]
            nc.sync.dma_start(
                out=allp[:].rearrange("p (r d) -> p r d", d=2),
                in_=agp_out[:].rearrange("(r p) d -> p r d", p=H))
            gsum = accp.tile([H, 1], f32, tag="pool", name="gsum")
            gmax = accp.tile([H, 1], f32, tag="pool", name="gmax")
            nc.vector.reduce_sum(
                out=gsum[:], in_=allp[:].rearrange("p (r d) -> p d r", d=2)[:, 0:1, :],
                axis=mybir.AxisListType.X)
            nc.vector.reduce_max(
                out=gmax[:], in_=allp[:].rearrange("p (r d) -> p d r", d=2)[:, 1:2, :],
                axis=mybir.AxisListType.X)
            nc.vector.tensor_scalar(out=gsum[:], in0=gsum[:], scalar1=1.0 / N,
                                    scalar2=None, op0=mybir.AluOpType.mult)
            pooled = accp.tile([2 * H, 1], f32, tag="pooled", name="pooled")
            nc.sync.dma_start(out=pooled[:H, :], in_=gsum[:])
            nc.sync.dma_start(out=pooled[H:, :], in_=gmax[:])
            h1ps = hdp.tile([H, 1], f32, tag="hd", name="h1ps")
            nc.tensor.matmul(out=h1ps[:], lhsT=fw1[:], rhs=pooled[:],
                             start=True, stop=True)
            r1 = accp.tile([H, 1], f32, tag="pool", name="r1")
            nc.scalar.activation(out=r1[:], in_=h1ps[:],
                                 func=mybir.ActivationFunctionType.Relu,
                                 bias=fb1[:, 0:1])
            h2ps = hdp.tile([1, 1], f32, tag="hd2", name="h2ps")
            nc.tensor.matmul(out=h2ps[:], lhsT=fw2[:], rhs=r1[:],
                             start=True, stop=True)
            ores = accp.tile([1, 1], f32, tag="ores", name="ores")
            nc.vector.tensor_add(out=ores[:], in0=h2ps[:],
                                 in1=fb2[:, 0:1])
            nc.sync.dma_start(out=out_d[:], in_=ores[:])

    nc.compile()
    return nc


def kernel(x, edge_index, W1, b1, W2, b2, W3, b3, fw1, fb1, fw2, fb2):
    dinv, order, core_of, slot_of, per_core = _preprocess(edge_index)

    core_inputs = []
    tiles_meta = None
    groups_meta = None
    idx_cols16 = mask_cols = None
    per_core_np = []
    for c in range(NC):
        xT, dinv_col, wrap, maskcat, groups, ncols = _build_core_inputs(
            x, dinv, order, per_core, c)
        per_core_np.append((xT, dinv_col, wrap, maskcat, groups, ncols))

    # All cores must share ONE program => unify tile/group metadata by padding
    # per-tile column counts to the max across cores.
    nts = np.zeros((NC, NT), np.int64)
    for c in range(NC):
        for t in range(NT):
            nts[c, t] = per_core[c][t][0]
    nt_max = nts.max(axis=0)

    # rebuild per-core inputs with unified nt per tile
    tiles_meta = [(int(nt_max[t]), None, None) for t in range(NT)]
    groups_meta = []
    g_tiles, g_cols = [], 0
    for t in range(NT):
        nt = int(nt_max[t])
        if g_tiles and g_cols + nt > MAXCOLS:
            groups_meta.append(g_tiles)
            g_tiles, g_cols = [], 0
        g_tiles.append(t)
        g_cols += nt
    if g_tiles:
        groups_meta.append(g_tiles)

    total_cols = int(nt_max.sum())
    idx_cols16 = total_cols * 8
    mask_cols = total_cols * 2

    in_maps = []
    b3col = np.stack([np.asarray(b1, np.float32), np.asarray(b2, np.float32),
                      np.asarray(b3, np.float32)], axis=1)  # [64, 3]
    ident = np.eye(128, dtype=np.float32)
    for c in range(NC):
        tiles = per_core[c]
        idx_flat = []
        mask_parts = []
        for g in groups_meta:
            for t in g:
                ntu = int(nt_max[t])
                if ntu == 0:
                    continue
                nt, idx, mask = tiles[t]
                idxu = np.zeros((ntu, 128), np.int16)
                masku = np.zeros((128, ntu, 2), np.float32)
                if nt:
                    idxu[:nt] = idx
                    masku[:, :nt, :] = mask
                idx_flat.append(idxu.reshape(-1))
                mask_parts.append(masku.reshape(128, ntu * 2))
        flat = np.concatenate(idx_flat)
        wrap = np.zeros((128, len(flat) // 16), np.int16)
        a = flat.reshape(-1, 16)
        for gg in range(8):
            wrap[gg * 16:(gg + 1) * 16, :] = a.T
        maskcat = np.concatenate(mask_parts, axis=1).astype(ml_dtypes.bfloat16)
        xT, dinv_col = per_core_np[c][0], per_core_np[c][1]
        in_maps.append({
            "xT": xT, "W1": np.asarray(W1, np.float32),
            "W2": np.asarray(W2, np.float32), "W3": np.asarray(W3, np.float32),
            "bcol": b3col, "dinvc": dinv_col, "idx16": wrap, "maskc": maskcat,
            "ident": ident, "fw1": np.asarray(fw1, np.float32),
            "fb1": np.asarray(fb1, np.float32).reshape(H, 1),
            "fw2": np.asarray(fw2, np.float32).reshape(H, 1),
            "fb2": np.asarray(fb2, np.float32).reshape(1, 1),
        })

    nc = _build_program(tiles_meta, groups_meta, idx_cols16, mask_cols)

    trace = os.environ.get("BASS_GCN_TRACE", "0") == "1"
    if trace:
        trace = _install_trace_hook()
    res = bass_utils.run_bass_kernel_spmd(
        nc, in_maps, core_ids=list(range(NC)), trace=trace)
    _EXEC_NS[0] = res.exec_time_ns
    out = res.results[0]["out"]
    return np.asarray(out, np.float32).reshape(1, 1)



# revision 14
# speedup vs baseline: 1.0188x; 1.0010x over previous
"""3-layer GCN + pooled MLP head on 8 Trainium2 NeuronCores.

Strategy (dst-sharded message passing):
- Relabel nodes by in-degree (desc), deal round-robin to 8 cores; each core
  owns 6250 dst nodes (padded to 6272 = 49 tiles of 128).
- Per layer: each core computes its slice of hhat = dinv * (y @ W) feature-major
  on PE, transposes to node-major, AllGathers hhat [50176, 64] to DRAM.
- Aggregation: per dst tile, dma_gather pulls 512B two-row blocks (int16 index
  = row//2) from DRAM hhat; a static 0/1 mask selects the right row parity and
  a strided DVE reduce does the per-node segment sum. Self-loop and dinv_dst
  scaling are fused in; bias+ReLU ride the PE transpose through the ACT engine.
- Head: per-core feature-major sum/max pools, tiny AllGather, replicated MLP.
"""
import os
import sys
import types

sys.path.insert(0, "/opt/trn_rl_repo")

import ml_dtypes
import numpy as np

import concourse.bass as bass
import concourse.bacc as bacc
import concourse.tile as tile
import concourse.mybir as mybir
from concourse import bass_utils

N = 50000
E = 800000
D_IN = 128
H = 64
NC = 8
NPC = 6272          # padded nodes per core (49 tiles of 128)
NT = 49             # dst tiles per core
NTOT = NC * NPC     # 50176 rows in the allgathered hhat
MAXCOLS = 48        # max block-columns per dma_gather call

_EXEC_NS = [None]


def _install_trace_hook():
    try:
        from trn_agent_boot.trn_boot import _ntff_profile_via_ctypes
        hook = _ntff_profile_via_ctypes('/opt/axon/libaxon_pjrt.so')
        if hook is None:
            return False
        mod = types.ModuleType('antenv.axon_hooks')
        mod.get_axon_ntff_profile_hook = lambda: hook
        sys.modules['antenv.axon_hooks'] = mod
        return True
    except Exception:
        return False


def _preprocess(edge_index):
    """Graph partitioning: relabel, shard, tile, build gather lists + masks."""
    src = np.asarray(edge_index[0], np.int64)
    dst = np.asarray(edge_index[1], np.int64)
    deg = np.bincount(dst, minlength=N)          # in-degree (no self loop)
    dinv = (1.0 / np.sqrt(deg + 1.0)).astype(np.float32)

    order = np.argsort(-deg, kind="stable")       # relabel: rank r -> orig order[r]
    rank_of = np.empty(N, np.int64)
    rank_of[order] = np.arange(N)
    core_of = rank_of % NC
    slot_of = rank_of // NC                       # 0..6249, degree-desc within core
    row_of = core_of * NPC + slot_of              # DRAM row in hhat_full

    per_core = []
    src_row = row_of[src]
    dst_core = core_of[dst]
    dst_slot = slot_of[dst]
    for c in range(NC):
        em = dst_core == c
        e_slot = dst_slot[em]
        e_srow = src_row[em]
        # neighbor lists per local slot
        o = np.argsort(e_slot, kind="stable")
        e_slot = e_slot[o]
        e_srow = e_srow[o]
        counts = np.bincount(e_slot, minlength=NPC)
        starts = np.concatenate([[0], np.cumsum(counts)])
        tiles = []
        for t in range(NT):
            sl0 = t * 128
            nt = int(counts[sl0:sl0 + 128].max()) if counts[sl0:sl0 + 128].size else 0
            if nt == 0:
                tiles.append((0, None, None))
                continue
            idx = np.zeros((nt, 128), np.int16)       # [col, part] block ids
            mask = np.zeros((128, nt, 2), np.float32)  # [part, col, parity]
            for p in range(128):
                s = sl0 + p
                if s >= NPC:
                    continue
                rows = e_srow[starts[s]:starts[s + 1]]
                k = len(rows)
                if k:
                    idx[:k, p] = (rows // 2).astype(np.int16)
                    mask[p, np.arange(k), (rows % 2)] = 1.0
            tiles.append((nt, idx, mask))
        per_core.append(tiles)
    return dinv, order, core_of, slot_of, per_core


def _build_core_inputs(x, dinv, order, per_core_tiles, c):
    """Per-core numpy inputs."""
    ranks = np.arange(c, N, NC)                  # global ranks owned by core c
    orig = order[ranks]                          # original node ids, slot order
    xT = np.zeros((D_IN, NPC), np.float32)
    xT[:, :len(orig)] = np.asarray(x, np.float32)[orig].T
    dv = np.zeros(NPC, np.float32)
    dv[:len(orig)] = dinv[orig]
    dinv_col = dv.reshape(NT, 128).T.copy()      # [128, 49]

    tiles = per_core_tiles[c]
    idx_parts, mask_parts, groups = [], [], []
    g_cols, g_tiles = 0, []
    for t in range(NT):
        nt = tiles[t][0]
        if g_tiles and g_cols + nt > MAXCOLS:
            groups.append(g_tiles)
            g_tiles, g_cols = [], 0
        g_tiles.append(t)
        g_cols += nt
    if g_tiles:
        groups.append(g_tiles)

    for g_tiles in groups:
        lst = []
        for t in g_tiles:
            nt, idx, mask = tiles[t]
            if nt:
                lst.append(idx.reshape(-1))       # [col, part] -> flat c*128+p
                mask_parts.append(mask.reshape(128, nt * 2))
        if lst:
            idx_parts.append(np.concatenate(lst))
    flat = np.concatenate(idx_parts) if idx_parts else np.zeros(128, np.int16)
    ncols_tot = len(flat) // 128
    wrap = np.zeros((128, len(flat) // 16), np.int16)
    a = flat.reshape(-1, 16)
    for gg in range(8):
        wrap[gg * 16:(gg + 1) * 16, :] = a.T
    maskcat = (np.concatenate(mask_parts, axis=1) if mask_parts
               else np.zeros((128, 2), np.float32))
    return xT, dinv_col, wrap, maskcat, groups, ncols_tot


def _build_program(tiles_meta, groups_meta, idx_cols16, mask_cols):
    """Build the bass program (same for all cores; per-core data via inputs)."""
    nc = bacc.Bacc("TRN2", target_bir_lowering=False, debug=False, num_devices=NC)
    f32 = mybir.dt.float32
    bf16 = mybir.dt.bfloat16
    xT_d = nc.dram_tensor("xT", [D_IN, NPC], f32, kind="ExternalInput")
    W1_d = nc.dram_tensor("W1", [D_IN, H], f32, kind="ExternalInput")
    W2_d = nc.dram_tensor("W2", [H, H], f32, kind="ExternalInput")
    W3_d = nc.dram_tensor("W3", [H, H], f32, kind="ExternalInput")
    bcol_d = nc.dram_tensor("bcol", [H, 3], f32, kind="ExternalInput")
    dinvc_d = nc.dram_tensor("dinvc", [128, NT], f32, kind="ExternalInput")
    idx_d = nc.dram_tensor("idx16", [128, idx_cols16], mybir.dt.int16, kind="ExternalInput")
    mask_d = nc.dram_tensor("maskc", [128, mask_cols], mybir.dt.bfloat16,
                            kind="ExternalInput")
    ident_d = nc.dram_tensor("ident", [128, 128], f32, kind="ExternalInput")
    fw1_d = nc.dram_tensor("fw1", [2 * H, H], f32, kind="ExternalInput")
    fb1_d = nc.dram_tensor("fb1", [H, 1], f32, kind="ExternalInput")
    fw2_d = nc.dram_tensor("fw2", [H, 1], f32, kind="ExternalInput")
    fb2_d = nc.dram_tensor("fb2", [1, 1], f32, kind="ExternalInput")
    out_d = nc.dram_tensor("out", [1, 1], f32, kind="ExternalOutput")

    Ws = [W1_d, W2_d, W3_d]
    NCHUNK = NPC // 512  # 12.25 -> handle per 512 with last 128-only tail via tiles

    with tile.TileContext(nc) as tc:
        with (
            tc.tile_pool(name="const", bufs=1) as cst,
            tc.tile_pool(name="hhat", bufs=1) as hhp,
            tc.tile_pool(name="yt", bufs=1) as ytp,
            tc.tile_pool(name="gb", bufs=2) as gbp,
            tc.tile_pool(name="acc", bufs=3) as accp,
            tc.tile_pool(name="ps", bufs=2, space="PSUM") as psp,
            tc.tile_pool(name="hdps", bufs=1, space="PSUM") as hdp,
            tc.tile_pool(name="zps", bufs=2, space="PSUM") as zpsp,
            tc.tile_pool(name="zsb", bufs=2) as zsbp,
            tc.tile_pool(name="dram", bufs=1, space="DRAM") as dram,
        ):
            # constants
            xT = cst.tile([D_IN, NPC], f32)
            nc.sync.dma_start(out=xT[:], in_=xT_d[:])
            W1 = cst.tile([D_IN, H], f32)
            nc.sync.dma_start(out=W1[:], in_=W1_d[:])
            W2 = cst.tile([H, H], f32)
            nc.sync.dma_start(out=W2[:], in_=W2_d[:])
            W3 = cst.tile([H, H], f32)
            nc.sync.dma_start(out=W3[:], in_=W3_d[:])
            Wt = [W1, W2, W3]
            bcol = cst.tile([H, 3], f32)
            nc.sync.dma_start(out=bcol[:], in_=bcol_d[:])
            dinvc = cst.tile([128, NT], f32)
            nc.sync.dma_start(out=dinvc[:], in_=dinvc_d[:])
            idx16 = cst.tile([128, idx_cols16], mybir.dt.int16)
            nc.sync.dma_start(out=idx16[:], in_=idx_d[:])
            maskc = cst.tile([128, mask_cols], bf16)
            nc.sync.dma_start(out=maskc[:], in_=mask_d[:])
            ident = cst.tile([128, 128], f32)
            nc.sync.dma_start(out=ident[:], in_=ident_d[:])
            fw1 = cst.tile([2 * H, H], f32)
            nc.sync.dma_start(out=fw1[:], in_=fw1_d[:])
            fb1 = cst.tile([H, 1], f32)
            nc.sync.dma_start(out=fb1[:], in_=fb1_d[:])
            fw2 = cst.tile([H, 1], f32)
            nc.sync.dma_start(out=fw2[:], in_=fw2_d[:])
            fb2 = cst.tile([1, 1], f32)
            nc.sync.dma_start(out=fb2[:], in_=fb2_d[:])

            hh2 = [hhp.tile([128, NT * H], f32, name="hhA"),
                   hhp.tile([128, NT * H], f32, name="hhB")]
            hb2 = [hhp.tile([128, NT * H], bf16, name="hbA"),
                   hhp.tile([128, NT * H], bf16, name="hbB")]
            yT = ytp.tile([H, NPC], f32)              # feature-major relu output

            ag_in = [dram.tile([NPC, H], bf16, name=f"agin{l}") for l in range(3)]
            ag_out = [dram.tile([NTOT, H], bf16, addr_space="Shared", name=f"agout{l}")
                      for l in range(3)]

            def mm_chunk(l, rhs_sb, ch0, cw):
                """One 512-col chunk of hhat_l = dinv*(W_l^T@rhs); chunked
                ag_in DMA so it overlaps the previous layer's aggregation."""
                hh, hb = hh2[l % 2], hb2[l % 2]
                zps = zpsp.tile([H, 512], f32, tag="zps")
                nc.tensor.matmul(out=zps[:, :cw], lhsT=Wt[l][:],
                                 rhs=rhs_sb[:, ch0:ch0 + cw],
                                 start=True, stop=True)
                zsb = zsbp.tile([H, 512], f32, tag="zsb")
                nc.vector.tensor_copy(out=zsb[:, :cw], in_=zps[:, :cw])
                for q in range(0, cw, 128):
                    t = (ch0 + q) // 128
                    tp = psp.tile([128, H], f32, tag="tp")
                    nc.tensor.transpose(out=tp[:], in_=zsb[:, q:q + 128],
                                        identity=ident[:H, :H])
                    nc.vector.tensor_scalar(
                        out=hh[:, t * H:(t + 1) * H], in0=tp[:],
                        scalar1=dinvc[:, t:t + 1], scalar2=None,
                        op0=mybir.AluOpType.mult)
                    nc.vector.tensor_copy(
                        out=hb[:, t * H:(t + 1) * H],
                        in_=hh[:, t * H:(t + 1) * H])
                t0, tw = ch0 // 128, cw // 128
                nc.sync.dma_start(
                    out=ag_in[l][ch0:ch0 + cw, :].rearrange("(t p) d -> p t d",
                                                            p=128),
                    in_=hb[:, t0 * H:(t0 + tw) * H].rearrange("p (t d) -> p t d",
                                                              d=H))

            def collective(l):
                nc.gpsimd.collective_compute(
                    "AllGather", mybir.AluOpType.bypass,
                    replica_groups=[list(range(NC))],
                    ins=[ag_in[l].opt()], outs=[ag_out[l].opt()])

            def make_fire(nl):
                """Fire next-layer mm chunks as soon as their yT tiles exist."""
                state = {"c": 0}
                nchunks = (NPC + 511) // 512

                def fire(t):
                    while (state["c"] < nchunks
                           and t >= min(4 * state["c"] + 3, NT - 1)):
                        c = state["c"]
                        ch0 = c * 512
                        mm_chunk(nl, yT, ch0, min(512, NPC - ch0))
                        state["c"] += 1
                return fire

            def aggregate(l, fire=None):
                """dst-tile aggregation from ag_out[l] into yT (or return y3)."""
                src_view = ag_out[l][:].rearrange("(a b) d -> a (b d)", b=2)
                cs_off = 0
                col_off = 0
                for g_tiles in groups_meta:
                    cols = sum(tiles_meta[t][0] for t in g_tiles)
                    if cols == 0:
                        continue
                    nidx = cols * 128
                    gb = gbp.tile([128, cols * 128], bf16, tag="gb",
                                  name=f"gb{l}_{g_tiles[0]}")
                    nc.gpsimd.dma_gather(
                        out_ap=gb[:].rearrange("p (n d) -> p n d", d=128),
                        in_ap=src_view,
                        idxs_ap=idx16[:, col_off * 8:(col_off + cols) * 8],
                        num_idxs=nidx, num_idxs_reg=nidx,
                        elem_size=128, single_packet=False)
                    seg = 0
                    for t in g_tiles:
                        nt = tiles_meta[t][0]
                        if nt == 0:
                            continue
                        gseg = gb[:, seg * 128:(seg + nt) * 128]
                        mseg = maskc[:, cs_off:cs_off + nt * 2]
                        nc.vector.tensor_tensor(
                            out=gseg.rearrange("p (cs d) -> p cs d", d=H),
                            in0=gseg.rearrange("p (cs d) -> p cs d", d=H),
                            in1=mseg.rearrange("p (cs u) -> p cs u", u=1)
                                .to_broadcast([128, nt * 2, H]),
                            op=mybir.AluOpType.mult)
                        acc = accp.tile([128, H], f32, tag="acc", name=f"acc{l}_{t}")
                        nc.vector.tensor_reduce(
                            out=acc[:],
                            in_=gseg.rearrange("p (cs d) -> p d cs", d=H),
                            axis=mybir.AxisListType.X, op=mybir.AluOpType.add)
                        nc.vector.tensor_add(out=acc[:], in0=acc[:],
                                             in1=hh2[l % 2][:, t * H:(t + 1) * H])
                        nc.vector.tensor_scalar(
                            out=acc[:], in0=acc[:], scalar1=dinvc[:, t:t + 1],
                            scalar2=None, op0=mybir.AluOpType.mult)
                        yps = psp.tile([H, 128], f32, tag="yps", name=f"yps{l}_{t}")
                        nc.tensor.transpose(out=yps[:], in_=acc[:], identity=ident[:])
                        nc.scalar.activation(
                            out=yT[:, t * 128:(t + 1) * 128], in_=yps[:],
                            func=mybir.ActivationFunctionType.Relu,
                            bias=bcol[:, l:l + 1])
                        if fire is not None:
                            fire(t)
                        cs_off += nt * 2
                        seg += nt
                    col_off += cols

            # ---- layer 1 ----
            for ch0 in range(0, NPC, 512):
                mm_chunk(0, xT, ch0, min(512, NPC - ch0))
            collective(0)
            aggregate(0, make_fire(1))
            # ---- layer 2 ----
            collective(1)
            aggregate(1, make_fire(2))
            # ---- layer 3 ----
            collective(2)
            aggregate(2)

            # zero pad columns of y3 (slots 6250..6271) before pooling
            nc.vector.memset(yT[:, NPC - 22:], 0.0)

            # pooling: sum and max over own nodes, feature-major
            sum_acc = accp.tile([H, 1], f32, tag="pool", name="sum_acc")
            max_acc = accp.tile([H, 1], f32, tag="pool", name="max_acc")
            first = True
            for ch0 in range(0, NPC, 512):
                cw = min(512, NPC - ch0)
                rs = accp.tile([H, 1], f32, tag="pool", name=f"rs{ch0}")
                rm = accp.tile([H, 1], f32, tag="pool", name=f"rm{ch0}")
                nc.vector.reduce_sum(out=rs[:], in_=yT[:, ch0:ch0 + cw],
                                     axis=mybir.AxisListType.X)
                nc.vector.reduce_max(out=rm[:], in_=yT[:, ch0:ch0 + cw],
                                     axis=mybir.AxisListType.X)
                if first:
                    nc.vector.tensor_copy(out=sum_acc[:], in_=rs[:])
                    nc.vector.tensor_copy(out=max_acc[:], in_=rm[:])
                    first = False
                else:
                    nc.vector.tensor_add(out=sum_acc[:], in0=sum_acc[:], in1=rs[:])
                    nc.vector.tensor_max(out=max_acc[:], in0=max_acc[:], in1=rm[:])

            pool2 = accp.tile([H, 2], f32, tag="pool", name="pool2")
            nc.vector.tensor_copy(out=pool2[:, 0:1], in_=sum_acc[:])
            nc.vector.tensor_copy(out=pool2[:, 1:2], in_=max_acc[:])
            agp_in = dram.tile([H, 2], f32, name="agpin")
            agp_out = dram.tile([NC * H, 2], f32, addr_space="Shared", name="agpout")
            nc.sync.dma_start(out=agp_in[:], in_=pool2[:])
            nc.gpsimd.collective_compute(
                "AllGather", mybir.AluOpType.bypass,
                replica_groups=[list(range(NC))],
                ins=[agp_in.opt()], outs=[agp_out.opt()])
            allp = accp.tile([H, 2 * NC], f32, tag="allp", name="allp")
            # rank r lands at cols 2r:2r+2 -> [64, 16]
            nc.sync.dma_start(
                out=allp[:].rearrange("p (r d) -> p r d", d=2),
                in_=agp_out[:].rearrange("(r p) d -> p r d", p=H))
            gsum = accp.tile([H, 1], f32, tag="pool", name="gsum")
            gmax = accp.tile([H, 1], f32, tag="pool", name="gmax")
            nc.vector.reduce_sum(
                out=gsum[:], in_=allp[:].rearrange("p (r d) -> p d r", d=2)[:, 0:1, :],
                axis=mybir.AxisListType.X)
            nc.vector.reduce_max(
                out=gmax[:], in_=allp[:].rearrange("p (r d) -> p d r", d=2)[:, 1:2, :],
                axis=mybir.AxisListType.X)
            nc.vector.tensor_scalar(out=gsum[:], in0=gsum[:], scalar1=1.0 / N,
                                    scalar2=None, op0=mybir.AluOpType.mult)
            pooled = accp.tile([2 * H, 1], f32, tag="pooled", name="pooled")
            nc.sync.dma_start(out=pooled[:H, :], in_=gsum[:])
            nc.sync.dma_start(out=pooled[H:, :], in_=gmax[:])
            h1ps = hdp.tile([H, 1], f32, tag="hd", name="h1ps")
            nc.tensor.matmul(out=h1ps[:], lhsT=fw1[:], rhs=pooled[:],
                             start=True, stop=True)
            r1 = accp.tile([H, 1], f32, tag="pool", name="r1")
            nc.scalar.activation(out=r1[:], in_=h1ps[:],
                                 func=mybir.ActivationFunctionType.Relu,
                                 bias=fb1[:, 0:1])
            h2ps = hdp.tile([1, 1], f32, tag="hd2", name="h2ps")
            nc.tensor.matmul(out=h2ps[:], lhsT=fw2[:], rhs=r1[:],
                             start=True, stop=True)
            ores = accp.tile([1, 1], f32, tag="ores", name="ores")
            nc.vector.tensor_add(out=ores[:], in0=h2ps[:],
                                 in1=fb2[:, 0:1])
            nc.sync.dma_start(out=out_d[:], in_=ores[:])

    nc.compile()
    return nc


def kernel(x, edge_index, W1, b1, W2, b2, W3, b3, fw1, fb1, fw2, fb2):
    dinv, order, core_of, slot_of, per_core = _preprocess(edge_index)

    core_inputs = []
    tiles_meta = None
    groups_meta = None
    idx_cols16 = mask_cols = None
    per_core_np = []
    for c in range(NC):
        xT, dinv_col, wrap, maskcat, groups, ncols = _build_core_inputs(
            x, dinv, order, per_core, c)
        per_core_np.append((xT, dinv_col, wrap, maskcat, groups, ncols))

    # All cores must share ONE program => unify tile/group metadata by padding
    # per-tile column counts to the max across cores.
    nts = np.zeros((NC, NT), np.int64)
    for c in range(NC):
        for t in range(NT):
            nts[c, t] = per_core[c][t][0]
    nt_max = nts.max(axis=0)

    # rebuild per-core inputs with unified nt per tile
    tiles_meta = [(int(nt_max[t]), None, None) for t in range(NT)]
    groups_meta = []
    g_tiles, g_cols = [], 0
    for t in range(NT):
        nt = int(nt_max[t])
        if g_tiles and g_cols + nt > MAXCOLS:
            groups_meta.append(g_tiles)
            g_tiles, g_cols = [], 0
        g_tiles.append(t)
        g_cols += nt
    if g_tiles:
        groups_meta.append(g_tiles)

    total_cols = int(nt_max.sum())
    idx_cols16 = total_cols * 8
    mask_cols = total_cols * 2

    in_maps = []
    b3col = np.stack([np.asarray(b1, np.float32), np.asarray(b2, np.float32),
                      np.asarray(b3, np.float32)], axis=1)  # [64, 3]
    ident = np.eye(128, dtype=np.float32)
    for c in range(NC):
        tiles = per_core[c]
        idx_flat = []
        mask_parts = []
        for g in groups_meta:
            for t in g:
                ntu = int(nt_max[t])
                if ntu == 0:
                    continue
                nt, idx, mask = tiles[t]
                idxu = np.zeros((ntu, 128), np.int16)
                masku = np.zeros((128, ntu, 2), np.float32)
                if nt:
                    idxu[:nt] = idx
                    masku[:, :nt, :] = mask
                idx_flat.append(idxu.reshape(-1))
                mask_parts.append(masku.reshape(128, ntu * 2))
        flat = np.concatenate(idx_flat)
        wrap = np.zeros((128, len(flat) // 16), np.int16)
        a = flat.reshape(-1, 16)
        for gg in range(8):
            wrap[gg * 16:(gg + 1) * 16, :] = a.T
        maskcat = np.concatenate(mask_parts, axis=1).astype(ml_dtypes.bfloat16)
        xT, dinv_col = per_core_np[c][0], per_core_np[c][1]
        in_maps.append({
            "xT": xT, "W1": np.asarray(W1, np.float32),
            "W2": np.asarray(W2, np.float32), "W3": np.asarray(W3, np.float32),
            "bcol": b3col, "dinvc": dinv_col, "idx16": wrap, "maskc": maskcat,
            "ident": ident, "fw1": np.asarray(fw1, np.float32),
            "fb1": np.asarray(fb1, np.float32).reshape(H, 1),
            "fw2": np.asarray(fw2, np.float32).reshape(H, 1),
            "fb2": np.asarray(fb2, np.float32).reshape(1, 1),
        })

    nc = _build_program(tiles_meta, groups_meta, idx_cols16, mask_cols)

    trace = os.environ.get("BASS_GCN_TRACE", "0") == "1"
    if trace:
        trace = _install_trace_hook()
    res = bass_utils.run_bass_kernel_spmd(
        nc, in_maps, core_ids=list(range(NC)), trace=trace)
    _EXEC_NS[0] = res.exec_time_ns
    out = res.results[0]["out"]
    return np.asarray(out, np.float32).reshape(1, 1)



# revision 17
# speedup vs baseline: 1.0246x; 1.0057x over previous
"""3-layer GCN + pooled MLP head on 8 Trainium2 NeuronCores.

Strategy (dst-sharded message passing):
- Relabel nodes by in-degree (desc), deal round-robin to 8 cores; each core
  owns 6250 dst nodes (padded to 6272 = 49 tiles of 128).
- Per layer: each core computes its slice of hhat = dinv * (y @ W) feature-major
  on PE, transposes to node-major, AllGathers hhat [50176, 64] to DRAM.
- Aggregation: per dst tile, dma_gather pulls 512B two-row blocks (int16 index
  = row//2) from DRAM hhat; a static 0/1 mask selects the right row parity and
  a strided DVE reduce does the per-node segment sum. Self-loop and dinv_dst
  scaling are fused in; bias+ReLU ride the PE transpose through the ACT engine.
- Head: per-core feature-major sum/max pools, tiny AllGather, replicated MLP.
"""
import os
import sys
import types

sys.path.insert(0, "/opt/trn_rl_repo")

import ml_dtypes
import numpy as np

import concourse.bass as bass
import concourse.bacc as bacc
import concourse.tile as tile
import concourse.mybir as mybir
from concourse import bass_utils

N = 50000
E = 800000
D_IN = 128
H = 64
NC = 8
NPC = 6272          # padded nodes per core (49 tiles of 128)
NT = 49             # dst tiles per core
NTOT = NC * NPC     # 50176 rows in the allgathered hhat
MAXCOLS = 48        # max block-columns per dma_gather call

_EXEC_NS = [None]


def _install_trace_hook():
    try:
        from trn_agent_boot.trn_boot import _ntff_profile_via_ctypes
        hook = _ntff_profile_via_ctypes('/opt/axon/libaxon_pjrt.so')
        if hook is None:
            return False
        mod = types.ModuleType('antenv.axon_hooks')
        mod.get_axon_ntff_profile_hook = lambda: hook
        sys.modules['antenv.axon_hooks'] = mod
        return True
    except Exception:
        return False


def _preprocess(edge_index):
    """Graph partitioning: relabel, shard, tile, build gather lists + masks."""
    src = np.asarray(edge_index[0], np.int64)
    dst = np.asarray(edge_index[1], np.int64)
    deg = np.bincount(dst, minlength=N)          # in-degree (no self loop)
    dinv = (1.0 / np.sqrt(deg + 1.0)).astype(np.float32)

    order = np.argsort(-deg, kind="stable")       # relabel: rank r -> orig order[r]
    rank_of = np.empty(N, np.int64)
    rank_of[order] = np.arange(N)
    core_of = rank_of % NC
    slot_of = rank_of // NC                       # 0..6249, degree-desc within core
    row_of = core_of * NPC + slot_of              # DRAM row in hhat_full

    per_core = []
    src_row = row_of[src]
    dst_core = core_of[dst]
    dst_slot = slot_of[dst]
    for c in range(NC):
        em = dst_core == c
        e_slot = dst_slot[em]
        e_srow = src_row[em]
        # neighbor lists per local slot
        o = np.argsort(e_slot, kind="stable")
        e_slot = e_slot[o]
        e_srow = e_srow[o]
        counts = np.bincount(e_slot, minlength=NPC)
        starts = np.concatenate([[0], np.cumsum(counts)])
        tiles = []
        for t in range(NT):
            sl0 = t * 128
            nt = int(counts[sl0:sl0 + 128].max()) if counts[sl0:sl0 + 128].size else 0
            if nt == 0:
                tiles.append((0, None, None))
                continue
            idx = np.zeros((nt, 128), np.int16)       # [col, part] block ids
            mask = np.zeros((128, nt, 2), np.float32)  # [part, col, parity]
            for p in range(128):
                s = sl0 + p
                if s >= NPC:
                    continue
                rows = e_srow[starts[s]:starts[s + 1]]
                k = len(rows)
                if k:
                    idx[:k, p] = (rows // 2).astype(np.int16)
                    mask[p, np.arange(k), (rows % 2)] = 1.0
            tiles.append((nt, idx, mask))
        per_core.append(tiles)
    return dinv, order, core_of, slot_of, per_core


def _build_core_inputs(x, dinv, order, per_core_tiles, c):
    """Per-core numpy inputs."""
    ranks = np.arange(c, N, NC)                  # global ranks owned by core c
    orig = order[ranks]                          # original node ids, slot order
    xT = np.zeros((D_IN, NPC), np.float32)
    xT[:, :len(orig)] = np.asarray(x, np.float32)[orig].T
    dv = np.zeros(NPC, np.float32)
    dv[:len(orig)] = dinv[orig]
    dinv_col = dv.reshape(NT, 128).T.copy()      # [128, 49]

    tiles = per_core_tiles[c]
    idx_parts, mask_parts, groups = [], [], []
    g_cols, g_tiles = 0, []
    for t in range(NT):
        nt = tiles[t][0]
        if g_tiles and g_cols + nt > MAXCOLS:
            groups.append(g_tiles)
            g_tiles, g_cols = [], 0
        g_tiles.append(t)
        g_cols += nt
    if g_tiles:
        groups.append(g_tiles)

    for g_tiles in groups:
        lst = []
        for t in g_tiles:
            nt, idx, mask = tiles[t]
            if nt:
                lst.append(idx.reshape(-1))       # [col, part] -> flat c*128+p
                mask_parts.append(mask.reshape(128, nt * 2))
        if lst:
            idx_parts.append(np.concatenate(lst))
    flat = np.concatenate(idx_parts) if idx_parts else np.zeros(128, np.int16)
    ncols_tot = len(flat) // 128
    wrap = np.zeros((128, len(flat) // 16), np.int16)
    a = flat.reshape(-1, 16)
    for gg in range(8):
        wrap[gg * 16:(gg + 1) * 16, :] = a.T
    maskcat = (np.concatenate(mask_parts, axis=1) if mask_parts
               else np.zeros((128, 2), np.float32))
    return xT, dinv_col, wrap, maskcat, groups, ncols_tot


def _build_program(tiles_meta, groups_meta, idx_cols16, mask_cols):
    """Build the bass program (same for all cores; per-core data via inputs)."""
    nc = bacc.Bacc("TRN2", target_bir_lowering=False, debug=False, num_devices=NC,
                   num_swdge_queues=2, dynamic_dma_scratch_size=24576)
    f32 = mybir.dt.float32
    bf16 = mybir.dt.bfloat16
    xT_d = nc.dram_tensor("xT", [D_IN, NPC], f32, kind="ExternalInput")
    W1_d = nc.dram_tensor("W1", [D_IN, H], f32, kind="ExternalInput")
    W2_d = nc.dram_tensor("W2", [H, H], f32, kind="ExternalInput")
    W3_d = nc.dram_tensor("W3", [H, H], f32, kind="ExternalInput")
    bcol_d = nc.dram_tensor("bcol", [H, 3], f32, kind="ExternalInput")
    dinvc_d = nc.dram_tensor("dinvc", [128, NT], f32, kind="ExternalInput")
    idx_d = nc.dram_tensor("idx16", [128, idx_cols16], mybir.dt.int16, kind="ExternalInput")
    mask_d = nc.dram_tensor("maskc", [128, mask_cols], mybir.dt.bfloat16,
                            kind="ExternalInput")
    ident_d = nc.dram_tensor("ident", [128, 128], f32, kind="ExternalInput")
    fw1_d = nc.dram_tensor("fw1", [2 * H, H], f32, kind="ExternalInput")
    fb1_d = nc.dram_tensor("fb1", [H, 1], f32, kind="ExternalInput")
    fw2_d = nc.dram_tensor("fw2", [H, 1], f32, kind="ExternalInput")
    fb2_d = nc.dram_tensor("fb2", [1, 1], f32, kind="ExternalInput")
    out_d = nc.dram_tensor("out", [1, 1], f32, kind="ExternalOutput")

    Ws = [W1_d, W2_d, W3_d]
    NCHUNK = NPC // 512  # 12.25 -> handle per 512 with last 128-only tail via tiles

    with tile.TileContext(nc) as tc:
        with (
            tc.tile_pool(name="const", bufs=1) as cst,
            tc.tile_pool(name="hhat", bufs=1) as hhp,
            tc.tile_pool(name="yt", bufs=1) as ytp,
            tc.tile_pool(name="gb", bufs=2) as gbp,
            tc.tile_pool(name="acc", bufs=3) as accp,
            tc.tile_pool(name="ps", bufs=2, space="PSUM") as psp,
            tc.tile_pool(name="hdps", bufs=1, space="PSUM") as hdp,
            tc.tile_pool(name="zps", bufs=2, space="PSUM") as zpsp,
            tc.tile_pool(name="zsb", bufs=2) as zsbp,
            tc.tile_pool(name="dram", bufs=1, space="DRAM") as dram,
        ):
            # constants
            xT = cst.tile([D_IN, NPC], f32)
            nc.sync.dma_start(out=xT[:], in_=xT_d[:])
            W1 = cst.tile([D_IN, H], f32)
            nc.sync.dma_start(out=W1[:], in_=W1_d[:])
            W2 = cst.tile([H, H], f32)
            nc.sync.dma_start(out=W2[:], in_=W2_d[:])
            W3 = cst.tile([H, H], f32)
            nc.sync.dma_start(out=W3[:], in_=W3_d[:])
            Wt = [W1, W2, W3]
            bcol = cst.tile([H, 3], f32)
            nc.sync.dma_start(out=bcol[:], in_=bcol_d[:])
            dinvc = cst.tile([128, NT], f32)
            nc.sync.dma_start(out=dinvc[:], in_=dinvc_d[:])
            idx16 = cst.tile([128, idx_cols16], mybir.dt.int16)
            nc.sync.dma_start(out=idx16[:], in_=idx_d[:])
            maskc = cst.tile([128, mask_cols], bf16)
            nc.sync.dma_start(out=maskc[:], in_=mask_d[:])
            ident = cst.tile([128, 128], f32)
            nc.sync.dma_start(out=ident[:], in_=ident_d[:])
            fw1 = cst.tile([2 * H, H], f32)
            nc.sync.dma_start(out=fw1[:], in_=fw1_d[:])
            fb1 = cst.tile([H, 1], f32)
            nc.sync.dma_start(out=fb1[:], in_=fb1_d[:])
            fw2 = cst.tile([H, 1], f32)
            nc.sync.dma_start(out=fw2[:], in_=fw2_d[:])
            fb2 = cst.tile([1, 1], f32)
            nc.sync.dma_start(out=fb2[:], in_=fb2_d[:])

            hh2 = [hhp.tile([128, NT * H], f32, name="hhA"),
                   hhp.tile([128, NT * H], f32, name="hhB")]
            hb2 = [hhp.tile([128, NT * H], bf16, name="hbA"),
                   hhp.tile([128, NT * H], bf16, name="hbB")]
            yT = ytp.tile([H, NPC], f32)              # feature-major relu output

            ag_in = [dram.tile([NPC, H], bf16, name=f"agin{l}") for l in range(3)]
            ag_out = [dram.tile([NTOT, H], bf16, addr_space="Shared", name=f"agout{l}")
                      for l in range(3)]

            def mm_chunk(l, rhs_sb, ch0, cw):
                """One 512-col chunk of hhat_l = dinv*(W_l^T@rhs); chunked
                ag_in DMA so it overlaps the previous layer's aggregation."""
                hh, hb = hh2[l % 2], hb2[l % 2]
                zps = zpsp.tile([H, 512], f32, tag="zps")
                nc.tensor.matmul(out=zps[:, :cw], lhsT=Wt[l][:],
                                 rhs=rhs_sb[:, ch0:ch0 + cw],
                                 start=True, stop=True)
                zsb = zsbp.tile([H, 512], f32, tag="zsb")
                nc.vector.tensor_copy(out=zsb[:, :cw], in_=zps[:, :cw])
                for q in range(0, cw, 128):
                    t = (ch0 + q) // 128
                    tp = psp.tile([128, H], f32, tag="tp")
                    nc.tensor.transpose(out=tp[:], in_=zsb[:, q:q + 128],
                                        identity=ident[:H, :H])
                    nc.vector.tensor_scalar(
                        out=hh[:, t * H:(t + 1) * H], in0=tp[:],
                        scalar1=dinvc[:, t:t + 1], scalar2=None,
                        op0=mybir.AluOpType.mult)
                    nc.vector.tensor_copy(
                        out=hb[:, t * H:(t + 1) * H],
                        in_=hh[:, t * H:(t + 1) * H])
                t0, tw = ch0 // 128, cw // 128
                nc.sync.dma_start(
                    out=ag_in[l][ch0:ch0 + cw, :].rearrange("(t p) d -> p t d",
                                                            p=128),
                    in_=hb[:, t0 * H:(t0 + tw) * H].rearrange("p (t d) -> p t d",
                                                              d=H))

            def collective(l):
                nc.gpsimd.collective_compute(
                    "AllGather", mybir.AluOpType.bypass,
                    replica_groups=[list(range(NC))],
                    ins=[ag_in[l].opt()], outs=[ag_out[l].opt()])

            def make_fire(nl):
                """Fire next-layer mm chunks as soon as their yT tiles exist."""
                state = {"c": 0}
                nchunks = (NPC + 511) // 512

                def fire(t):
                    while (state["c"] < nchunks
                           and t >= min(4 * state["c"] + 3, NT - 1)):
                        c = state["c"]
                        ch0 = c * 512
                        mm_chunk(nl, yT, ch0, min(512, NPC - ch0))
                        state["c"] += 1
                return fire

            def aggregate(l, fire=None):
                """dst-tile aggregation from ag_out[l] into yT (or return y3)."""
                src_view = ag_out[l][:].rearrange("(a b) d -> a (b d)", b=2)
                cs_off = 0
                col_off = 0
                gi = 0
                for g_tiles in groups_meta:
                    cols = sum(tiles_meta[t][0] for t in g_tiles)
                    if cols == 0:
                        continue
                    nidx = cols * 128
                    gb = gbp.tile([128, cols * 128], bf16, tag="gb",
                                  name=f"gb{l}_{g_tiles[0]}")
                    nc.gpsimd.dma_gather(
                        out_ap=gb[:].rearrange("p (n d) -> p n d", d=128),
                        in_ap=src_view,
                        idxs_ap=idx16[:, col_off * 8:(col_off + cols) * 8],
                        num_idxs=nidx, num_idxs_reg=nidx,
                        elem_size=128, single_packet=False, queue_num=gi % 2)
                    gi += 1
                    seg = 0
                    for t in g_tiles:
                        nt = tiles_meta[t][0]
                        if nt == 0:
                            continue
                        gseg = gb[:, seg * 128:(seg + nt) * 128]
                        mseg = maskc[:, cs_off:cs_off + nt * 2]
                        nc.vector.tensor_tensor(
                            out=gseg.rearrange("p (cs d) -> p cs d", d=H),
                            in0=gseg.rearrange("p (cs d) -> p cs d", d=H),
                            in1=mseg.rearrange("p (cs u) -> p cs u", u=1)
                                .to_broadcast([128, nt * 2, H]),
                            op=mybir.AluOpType.mult)
                        acc = accp.tile([128, H], f32, tag="acc", name=f"acc{l}_{t}")
                        nc.vector.tensor_reduce(
                            out=acc[:],
                            in_=gseg.rearrange("p (cs d) -> p d cs", d=H),
                            axis=mybir.AxisListType.X, op=mybir.AluOpType.add)
                        nc.vector.tensor_add(out=acc[:], in0=acc[:],
                                             in1=hh2[l % 2][:, t * H:(t + 1) * H])
                        nc.vector.tensor_scalar(
                            out=acc[:], in0=acc[:], scalar1=dinvc[:, t:t + 1],
                            scalar2=None, op0=mybir.AluOpType.mult)
                        yps = psp.tile([H, 128], f32, tag="yps", name=f"yps{l}_{t}")
                        nc.tensor.transpose(out=yps[:], in_=acc[:], identity=ident[:])
                        nc.scalar.activation(
                            out=yT[:, t * 128:(t + 1) * 128], in_=yps[:],
                            func=mybir.ActivationFunctionType.Relu,
                            bias=bcol[:, l:l + 1])
                        if fire is not None:
                            fire(t)
                        cs_off += nt * 2
                        seg += nt
                    col_off += cols

            # ---- layer 1 ----
            for ch0 in range(0, NPC, 512):
                mm_chunk(0, xT, ch0, min(512, NPC - ch0))
            collective(0)
            aggregate(0, make_fire(1))
            # ---- layer 2 ----
            collective(1)
            aggregate(1, make_fire(2))
            # ---- layer 3 ----
            collective(2)
            aggregate(2)

            # zero pad columns of y3 (slots 6250..6271) before pooling
            nc.vector.memset(yT[:, NPC - 22:], 0.0)

            # pooling: sum and max over own nodes, feature-major
            sum_acc = accp.tile([H, 1], f32, tag="pool", name="sum_acc")
            max_acc = accp.tile([H, 1], f32, tag="pool", name="max_acc")
            first = True
            for ch0 in range(0, NPC, 512):
                cw = min(512, NPC - ch0)
                rs = accp.tile([H, 1], f32, tag="pool", name=f"rs{ch0}")
                rm = accp.tile([H, 1], f32, tag="pool", name=f"rm{ch0}")
                nc.vector.reduce_sum(out=rs[:], in_=yT[:, ch0:ch0 + cw],
                                     axis=mybir.AxisListType.X)
                nc.vector.reduce_max(out=rm[:], in_=yT[:, ch0:ch0 + cw],
                                     axis=mybir.AxisListType.X)
                if first:
                    nc.vector.tensor_copy(out=sum_acc[:], in_=rs[:])
                    nc.vector.tensor_copy(out=max_acc[:], in_=rm[:])
                    first = False
                else:
                    nc.vector.tensor_add(out=sum_acc[:], in0=sum_acc[:], in1=rs[:])
                    nc.vector.tensor_max(out=max_acc[:], in0=max_acc[:], in1=rm[:])

            pool2 = accp.tile([H, 2], f32, tag="pool", name="pool2")
            nc.vector.tensor_copy(out=pool2[:, 0:1], in_=sum_acc[:])
            nc.vector.tensor_copy(out=pool2[:, 1:2], in_=max_acc[:])
            agp_in = dram.tile([H, 2], f32, name="agpin")
            agp_out = dram.tile([NC * H, 2], f32, addr_space="Shared", name="agpout")
            nc.sync.dma_start(out=agp_in[:], in_=pool2[:])
            nc.gpsimd.collective_compute(
                "AllGather", mybir.AluOpType.bypass,
                replica_groups=[list(range(NC))],
                ins=[agp_in.opt()], outs=[agp_out.opt()])
            allp = accp.tile([H, 2 * NC], f32, tag="allp", name="allp")
            # rank r lands at cols 2r:2r+2 -> [64, 16]
            nc.sync.dma_start(
                out=allp[:].rearrange("p (r d) -> p r d", d=2),
                in_=agp_out[:].rearrange("(r p) d -> p r d", p=H))
            gsum = accp.tile([H, 1], f32, tag="pool", name="gsum")
            gmax = accp.tile([H, 1], f32, tag="pool", name="gmax")
            nc.vector.reduce_sum(
                out=gsum[:], in_=allp[:].rearrange("p (r d) -> p d r", d=2)[:, 0:1, :],
                axis=mybir.AxisListType.X)
            nc.vector.reduce_max(
                out=gmax[:], in_=allp[:].rearrange("p (r d) -> p d r", d=2)[:, 1:2, :],
                axis=mybir.AxisListType.X)
            nc.vector.tensor_scalar(out=gsum[:], in0=gsum[:], scalar1=1.0 / N,
                                    scalar2=None, op0=mybir.AluOpType.mult)
            pooled = accp.tile([2 * H, 1], f32, tag="pooled", name="pooled")
            nc.sync.dma_start(out=pooled[:H, :], in_=gsum[:])
            nc.sync.dma_start(out=pooled[H:, :], in_=gmax[:])
            h1ps = hdp.tile([H, 1], f32, tag="hd", name="h1ps")
            nc.tensor.matmul(out=h1ps[:], lhsT=fw1[:], rhs=pooled[:],
                             start=True, stop=True)
            r1 = accp.tile([H, 1], f32, tag="pool", name="r1")
            nc.scalar.activation(out=r1[:], in_=h1ps[:],
                                 func=mybir.ActivationFunctionType.Relu,
                                 bias=fb1[:, 0:1])
            h2ps = hdp.tile([1, 1], f32, tag="hd2", name="h2ps")
            nc.tensor.matmul(out=h2ps[:], lhsT=fw2[:], rhs=r1[:],
                             start=True, stop=True)
            ores = accp.tile([1, 1], f32, tag="ores", name="ores")
            nc.vector.tensor_add(out=ores[:], in0=h2ps[:],
                                 in1=fb2[:, 0:1])
            nc.sync.dma_start(out=out_d[:], in_=ores[:])

    nc.compile()
    return nc


def kernel(x, edge_index, W1, b1, W2, b2, W3, b3, fw1, fb1, fw2, fb2):
    dinv, order, core_of, slot_of, per_core = _preprocess(edge_index)

    core_inputs = []
    tiles_meta = None
    groups_meta = None
    idx_cols16 = mask_cols = None
    per_core_np = []
    for c in range(NC):
        xT, dinv_col, wrap, maskcat, groups, ncols = _build_core_inputs(
            x, dinv, order, per_core, c)
        per_core_np.append((xT, dinv_col, wrap, maskcat, groups, ncols))

    # All cores must share ONE program => unify tile/group metadata by padding
    # per-tile column counts to the max across cores.
    nts = np.zeros((NC, NT), np.int64)
    for c in range(NC):
        for t in range(NT):
            nts[c, t] = per_core[c][t][0]
    nt_max = nts.max(axis=0)

    # rebuild per-core inputs with unified nt per tile
    tiles_meta = [(int(nt_max[t]), None, None) for t in range(NT)]
    groups_meta = []
    g_tiles, g_cols = [], 0
    for t in range(NT):
        nt = int(nt_max[t])
        if g_tiles and g_cols + nt > MAXCOLS:
            groups_meta.append(g_tiles)
            g_tiles, g_cols = [], 0
        g_tiles.append(t)
        g_cols += nt
    if g_tiles:
        groups_meta.append(g_tiles)

    total_cols = int(nt_max.sum())
    idx_cols16 = total_cols * 8
    mask_cols = total_cols * 2

    in_maps = []
    b3col = np.stack([np.asarray(b1, np.float32), np.asarray(b2, np.float32),
                      np.asarray(b3, np.float32)], axis=1)  # [64, 3]
    ident = np.eye(128, dtype=np.float32)
    for c in range(NC):
        tiles = per_core[c]
        idx_flat = []
        mask_parts = []
        for g in groups_meta:
            for t in g:
                ntu = int(nt_max[t])
                if ntu == 0:
                    continue
                nt, idx, mask = tiles[t]
                idxu = np.zeros((ntu, 128), np.int16)
                masku = np.zeros((128, ntu, 2), np.float32)
                if nt:
                    idxu[:nt] = idx
                    masku[:, :nt, :] = mask
                idx_flat.append(idxu.reshape(-1))
                mask_parts.append(masku.reshape(128, ntu * 2))
        flat = np.concatenate(idx_flat)
        wrap = np.zeros((128, len(flat) // 16), np.int16)
        a = flat.reshape(-1, 16)
        for gg in range(8):
            wrap[gg * 16:(gg + 1) * 16, :] = a.T
        maskcat = np.concatenate(mask_parts, axis=1).astype(ml_dtypes.bfloat16)
        xT, dinv_col = per_core_np[c][0], per_core_np[c][1]
        in_maps.append({
            "xT": xT, "W1": np.asarray(W1, np.float32),
            "W2": np.asarray(W2, np.float32), "W3": np.asarray(W3, np.float32),
            "bcol": b3col, "dinvc": dinv_col, "idx16": wrap, "maskc": maskcat,
            "ident": ident, "fw1": np.asarray(fw1, np.float32),
            "fb1": np.asarray(fb1, np.float32).reshape(H, 1),
            "fw2": np.asarray(fw2, np.float32).reshape(H, 1),
            "fb2": np.asarray(fb2, np.float32).reshape(1, 1),
        })

    nc = _build_program(tiles_meta, groups_meta, idx_cols16, mask_cols)

    trace = os.environ.get("BASS_GCN_TRACE", "0") == "1"
    if trace:
        trace = _install_trace_hook()
    res = bass_utils.run_bass_kernel_spmd(
        nc, in_maps, core_ids=list(range(NC)), trace=trace)
    _EXEC_NS[0] = res.exec_time_ns
    out = res.results[0]["out"]
    return np.asarray(out, np.float32).reshape(1, 1)

